# revision 1
# baseline (speedup 1.0000x reference)
"""Trainium2 Bass kernel for a dense transformer block (RMSNorm -> QKV+RoPE ->
attention -> proj -> RMSNorm -> SiLU FFN), sharded over 8 NeuronCores.

Sharding: token-split. Core c handles batch b=c//4 and query tokens
[qo:qo+512) of that batch (qo=(c%4)*512). Each core computes K/V for its
whole batch (replicated x4) so no collectives are needed. Host feeds each
core its batch's z_H/z_L *transposed* ([D, T]) with the core's own 512
tokens permuted to the front, so one SPMD program serves all cores.

Dataflow is kept transposed ([D, tok] on chip) so every matmul contracts
over the partition axis directly. Weights are fed bf16 with the RMSNorm
gains folded in on host; softmax/norm statistics stay fp32. Softmax skips
max-subtraction (scores are O(+-5) by construction) and gets its
denominator from a ones-column appended to V.
"""

import math
from contextlib import ExitStack

import ml_dtypes
import numpy as np

import concourse.bass as bass
from concourse import bacc
import concourse.mybir as mybir
import concourse.tile as tile
from concourse.bass_utils import run_bass_kernel_spmd
from concourse.masks import make_identity

FP32 = mybir.dt.float32
BF16 = mybir.dt.bfloat16
AF = mybir.ActivationFunctionType

B, S, D, F, H, DH = 2, 2048, 1024, 4096, 16, 64
HALF = DH // 2
NCORES = 8
CPB = NCORES // B  # cores per batch
QN = S // CPB  # own query tokens per core
EPS = 1e-6
ROPE_BASE = 10000.0
P = 128


def build_bass(T=S, Qn=QN, D_=D, F_=F):
    """Emit the per-core program. All cores run this same NEFF."""
    KD = D_ // P  # hidden-dim partition chunks
    KF = F_ // P
    TT = T // P  # token tiles (batch)
    QT = Qn // P  # token tiles (own)
    W = min(512, D_)  # matmul moving-dim window
    HPW = W // DH  # heads per window
    QW = min(256, Qn)  # attention query window
    NQW = Qn // QW
    nheads = D_ // DH

    nc = bacc.Bacc()
    zz = nc.dram_tensor("zz", [D_, 2, T], FP32, kind="ExternalInput")
    wqkv = nc.dram_tensor("wqkv", [D_, 3 * D_], BF16, kind="ExternalInput")
    wproj = nc.dram_tensor("wproj", [D_, D_], BF16, kind="ExternalInput")
    wf1 = nc.dram_tensor("wf1", [D_, F_], BF16, kind="ExternalInput")
    wf2 = nc.dram_tensor("wf2", [F_, D_], BF16, kind="ExternalInput")
    cs = nc.dram_tensor("cs", [T, 2 * HALF], FP32, kind="ExternalInput")
    outd = nc.dram_tensor("outt", [D_, Qn], FP32, kind="ExternalOutput")

    with tile.TileContext(nc) as tc:
        with ExitStack() as ctx:
            pool = lambda name, bufs, **kw: ctx.enter_context(tc.tile_pool(name=name, bufs=bufs, **kw))
            p1a = pool("p1m_a", 2)      # zz halves / expT
            p1b = pool("p1m_b", 2)      # weight streams
            pxt = pool("pxt", 2)        # x scratch halves
            phalf = pool("phalf", 2)    # sq / roped qk
            biga = pool("big_a", 1)     # hiddenT -> siluT
            bigb = pool("big_b", 1)     # kT
            bigc = pool("big_c", 1)     # v65
            p1c = pool("p1m_c", 1)      # qT -> h2T
            p1d = pool("p1m_d", 1)      # attnT
            pxq = pool("pxq", 1)        # xqT / x2T
            prstd = pool("prstd", 1)    # rstd_rep
            prope = pool("prope", 2)    # cos/sin rep + tmps
            prow = pool("prow", 1)      # small rows
            phead = pool("phead", 2)    # per-head rows
            pout = pool("pout", 1)      # output staging
            pwstr = pool("pwstr", 2)    # ffn1 weight double-buffer
            psingle = pool("psingle", 1)  # constants
            ps_mm = pool("ps_mm", 3, space="PSUM")
            ps_stats = pool("ps_stats", 1, space="PSUM")
            ps_tp = pool("ps_tp", 1, space="PSUM")

            ones_col = psingle.tile([P, 1], BF16)
            nc.vector.memset(ones_col, 1.0)
            ones_row = psingle.tile([1, P], FP32)
            nc.vector.memset(ones_row, 1.0)
            ident = psingle.tile([P, P], BF16)
            make_identity(nc, ident)
            eps_t = psingle.tile([P, 1], FP32)
            nc.vector.memset(eps_t, EPS)
            zero_t = psingle.tile([P, 1], FP32)
            nc.vector.memset(zero_t, 0.0)

            # ---- pass 1: x = zh + zl, accumulate sum(x^2) over D ----
            TH = min(1024, T)
            NTH = T // TH
            xq = pxq.tile([P, KD, Qn], FP32, tag="xq")
            st1 = ps_stats.tile([1, T], FP32, tag="stps")
            for dc in range(KD):
                for th in range(NTH):
                    t0 = th * TH
                    zt = p1a.tile([P, 2, TH], FP32, tag="t1m_a")
                    nc.gpsimd.dma_start(zt, zz[dc * P : (dc + 1) * P, :, t0 : t0 + TH])
                    xt = pxt.tile([P, TH], FP32, tag="xt")
                    nc.vector.tensor_add(xt, zt[:, 0, :], zt[:, 1, :])
                    if t0 < Qn:
                        qe = min(Qn - t0, TH)
                        nc.vector.tensor_copy(xq[:, dc, 0:qe], xt[:, 0:qe])
                    sq = phalf.tile([P, TH], BF16, tag="thalf")
                    nc.vector.tensor_mul(sq, xt, xt)
                    for nw in range(TH // W):
                        nc.tensor.matmul(
                            st1[0:1, t0 + nw * W : t0 + (nw + 1) * W],
                            ones_col,
                            sq[:, nw * W : (nw + 1) * W],
                            start=(dc == 0),
                            stop=(dc == KD - 1),
                        )

            # rstd row + broadcast to all partitions via K=1 matmul
            rows1 = prow.tile([33, T], FP32, tag="srow")
            nc.scalar.activation(rows1[32:33, :], st1[0:1, :], AF.Sqrt, bias=eps_t[32:33], scale=1.0 / D_)
            nc.vector.reciprocal(rows1[0:1, :], rows1[32:33, :])
            rstd = prstd.tile([P, T], BF16, tag="rstd")
            for nw in range(T // W):
                rb = ps_stats.tile([P, W], FP32, tag="stps")
                nc.tensor.matmul(
                    rb, ones_row, rows1[0:1, nw * W : (nw + 1) * W], start=True, stop=True
                )
                nc.vector.tensor_copy(rstd[:, nw * W : (nw + 1) * W], rb)

            # ---- pass 2: hiddenT = (zh + zl) * rstd (bf16) ----
            hid = biga.tile([P, KD, T], BF16, tag="big_a")
            for dc in range(KD):
                for th in range(NTH):
                    t0 = th * TH
                    zt = p1a.tile([P, 2, TH], FP32, tag="t1m_a")
                    nc.gpsimd.dma_start(zt, zz[dc * P : (dc + 1) * P, :, t0 : t0 + TH])
                    xt = pxt.tile([P, TH], FP32, tag="xt")
                    nc.vector.tensor_add(xt, zt[:, 0, :], zt[:, 1, :])
                    nc.vector.tensor_mul(hid[:, dc, t0 : t0 + TH], xt, rstd[:, t0 : t0 + TH])

            # ---- QKV projections ----
            kT = bigb.tile([P, KD, T], BF16, tag="big_b")
            qT = p1c.tile([P, KD, Qn], BF16, tag="t1m_c")
            v65 = bigc.tile([P, TT, nheads, DH + 1], BF16, tag="big_c")
            nc.vector.memset(v65[:, :, :, DH : DH + 1], 1.0)

            NW3 = 3 * D_ // W
            NWQ = D_ // W  # windows for q (same count for k, v)
            for cw in range(NW3):
                wt = p1b.tile([P, KD, W], BF16, tag="t1m_b")
                nc.sync.dma_start(
                    wt, wqkv[:, cw * W : (cw + 1) * W].rearrange("(c p) w -> p c w", p=P)
                )
                is_q = cw < NWQ
                is_v = cw >= 2 * NWQ
                ntok = QT if is_q else TT
                for tt in range(ntok):
                    ps = ps_mm.tile([P, W], FP32, tag="mmps")
                    for dc in range(KD):
                        nc.tensor.matmul(
                            ps,
                            hid[:, dc, tt * P : (tt + 1) * P],
                            wt[:, dc, :],
                            start=(dc == 0),
                            stop=(dc == KD - 1),
                        )
                    ps3 = ps.rearrange("p (h j) -> p h j", j=DH)
                    if is_v:
                        h0 = (cw - 2 * NWQ) * HPW
                        nc.vector.tensor_copy(
                            v65[:, tt, h0 : h0 + HPW, 0:DH], ps3
                        )
                    else:
                        csrep = prope.tile([P, HPW, 2 * HALF], FP32, tag="crep")
                        cna = cs[tt * P : (tt + 1) * P, :]
                        nc.sync.dma_start(
                            csrep,
                            bass.AP(
                                tensor=cna.tensor,
                                offset=cna.offset,
                                ap=[list(cna.ap[0]), [0, HPW], list(cna.ap[1])],
                            ),
                        )
                        crep = csrep[:, :, 0:HALF]
                        srep = csrep[:, :, HALF : 2 * HALF]
                        rop = phalf.tile([P, W], BF16, tag="thalf")
                        rop3 = rop.rearrange("p (h j) -> p h j", j=DH)
                        ta = prope.tile([P, HPW, HALF], BF16, tag="ta")
                        tb = prope.tile([P, HPW, HALF], BF16, tag="tb")
                        nc.vector.tensor_mul(ta, ps3[:, :, 0:HALF], crep)
                        nc.vector.tensor_mul(tb, ps3[:, :, HALF:DH], srep)
                        nc.vector.tensor_sub(rop3[:, :, 0:HALF], ta, tb)
                        tc2 = prope.tile([P, HPW, HALF], BF16, tag="ta")
                        td = prope.tile([P, HPW, HALF], BF16, tag="tb")
                        nc.vector.tensor_mul(tc2, ps3[:, :, HALF:DH], crep)
                        nc.vector.tensor_mul(td, ps3[:, :, 0:HALF], srep)
                        nc.vector.tensor_add(rop3[:, :, HALF:DH], tc2, td)
                        # transpose roped tile into qT / kT
                        for c2 in range(W // P):
                            tp = ps_tp.tile([P, P], BF16, tag="tpps")
                            nc.tensor.transpose(
                                tp, rop[:, c2 * P : (c2 + 1) * P], ident
                            )
                            if is_q:
                                gc = cw * (W // P) + c2
                                nc.vector.tensor_copy(
                                    qT[:, gc, tt * P : (tt + 1) * P], tp
                                )
                            else:
                                gc = (cw - NWQ) * (W // P) + c2
                                nc.vector.tensor_copy(
                                    kT[:, gc, tt * P : (tt + 1) * P], tp
                                )

            # ---- attention (scores kept transposed: [ktok, qtok]) ----
            attn = p1d.tile([P, KD, Qn], BF16, tag="t1m_d")
            for h in range(nheads):
                hc, hp = h // 2, (h % 2) * DH
                for qw in range(NQW):
                    qsl = qT[hp : hp + DH, hc, qw * QW : (qw + 1) * QW]
                    ex = p1a.tile([P, TT, QW], BF16, tag="t1m_a")
                    for kt in range(TT):
                        pss = ps_mm.tile([P, QW], FP32, tag="mmps")
                        nc.tensor.matmul(
                            pss,
                            kT[hp : hp + DH, hc, kt * P : (kt + 1) * P],
                            qsl,
                            start=True,
                            stop=True,
                        )
                        nc.scalar.activation(
                            ex[:, kt, :], pss, AF.Exp, bias=zero_t, scale=1.0 / math.sqrt(DH)
                        )
                    pso = ps_mm.tile([DH + 1, QW], FP32, tag="mmps")
                    for kt in range(TT):
                        nc.tensor.matmul(
                            pso,
                            v65[:, kt, h, :],
                            ex[:, kt, :],
                            start=(kt == 0),
                            stop=(kt == TT - 1),
                        )
                    rc = phead.tile([1, QW], FP32, tag="rcrow")
                    nc.vector.reciprocal(rc, pso[DH : DH + 1, :])
                    rb = ps_tp.tile([DH, QW], FP32, tag="tpps")
                    nc.tensor.matmul(rb, ones_row[0:1, 0:DH], rc, start=True, stop=True)
                    rbs = phead.tile([DH, QW], FP32, tag="rbsb")
                    nc.vector.tensor_copy(rbs, rb)
                    nc.vector.tensor_mul(
                        attn[hp : hp + DH, hc, qw * QW : (qw + 1) * QW],
                        pso[0:DH, :],
                        rbs,
                    )

            # ---- proj + residual (x2T accumulated into xq in place) ----
            for dt in range(KD):
                wp = p1b.tile([P, KD, P], BF16, tag="t1m_b")
                nc.sync.dma_start(
                    wp, wproj[:, dt * P : (dt + 1) * P].rearrange("(c p) m -> p c m", p=P)
                )
                ps = ps_mm.tile([P, Qn], FP32, tag="mmps")
                for ac in range(KD):
                    nc.tensor.matmul(
                        ps, wp[:, ac, :], attn[:, ac, :], start=(ac == 0), stop=(ac == KD - 1)
                    )
                nc.vector.tensor_add(xq[:, dt, :], ps, xq[:, dt, :])

            # ---- norm2 ----
            st2 = ps_stats.tile([1, Qn], FP32, tag="stps")
            for dt in range(KD):
                sq2 = phalf.tile([P, Qn], BF16, tag="thalf")
                nc.vector.tensor_mul(sq2, xq[:, dt, :], xq[:, dt, :])
                for nw in range(Qn // W if Qn >= W else 1):
                    w0 = nw * min(W, Qn)
                    w1 = min(w0 + W, Qn)
                    nc.tensor.matmul(
                        st2[0:1, w0:w1],
                        ones_col,
                        sq2[:, w0:w1],
                        start=(dt == 0),
                        stop=(dt == KD - 1),
                    )
            rows2 = prow.tile([33, Qn], FP32, tag="srow")
            nc.scalar.activation(rows2[32:33, :], st2[0:1, :], AF.Sqrt, bias=eps_t[32:33], scale=1.0 / D_)
            nc.vector.reciprocal(rows2[0:1, :], rows2[32:33, :])
            rstd2 = prstd.tile([P, Qn], BF16, tag="rstd")
            for nw in range(max(1, Qn // W)):
                w0 = nw * min(W, Qn)
                w1 = min(w0 + W, Qn)
                rb2 = ps_stats.tile([P, min(W, Qn)], FP32, tag="stps")
                nc.tensor.matmul(rb2, ones_row, rows2[0:1, w0:w1], start=True, stop=True)
                nc.vector.tensor_copy(rstd2[:, w0:w1], rb2)
            h2 = p1c.tile([P, KD, Qn], BF16, tag="t1m_c")
            for dt in range(KD):
                nc.vector.tensor_mul(h2[:, dt, :], xq[:, dt, :], rstd2)

            # ---- FFN ----
            sil = biga.tile([P, KF, Qn], BF16, tag="big_a")
            for ft in range(KF):
                w1t = pwstr.tile([P, KD, P], BF16, tag="w1t")
                nc.sync.dma_start(
                    w1t, wf1[:, ft * P : (ft + 1) * P].rearrange("(c p) m -> p c m", p=P)
                )
                ps = ps_mm.tile([P, Qn], FP32, tag="mmps")
                for dc in range(KD):
                    nc.tensor.matmul(
                        ps, w1t[:, dc, :], h2[:, dc, :], start=(dc == 0), stop=(dc == KD - 1)
                    )
                nc.scalar.activation(sil[:, ft, :], ps, AF.Silu, bias=zero_t)
            for dt in range(KD):
                w2t = p1b.tile([P, KF, P], BF16, tag="t1m_b")
                nc.sync.dma_start(
                    w2t, wf2[:, dt * P : (dt + 1) * P].rearrange("(c p) m -> p c m", p=P)
                )
                ps = ps_mm.tile([P, Qn], FP32, tag="mmps")
                for fc in range(KF):
                    nc.tensor.matmul(
                        ps, w2t[:, fc, :], sil[:, fc, :], start=(fc == 0), stop=(fc == KF - 1)
                    )
                ot = pout.tile([P, Qn], FP32, tag="outsb")
                nc.vector.tensor_add(ot, ps, xq[:, dt, :])
                nc.sync.dma_start(outd[dt * P : (dt + 1) * P, :], ot)

    nc.finalize()
    return nc


def _rope_tables(T):
    inv = ROPE_BASE ** (-np.arange(HALF, dtype=np.float64) / HALF)
    fr = np.arange(T, dtype=np.float64)[:, None] * inv[None, :]
    return np.cos(fr).astype(np.float32), np.sin(fr).astype(np.float32)


def make_in_maps(z_H, z_L, w_qkv, w_proj, w_ffn1, w_ffn2, g1, g2, T=S, Qn=QN, ncores=NCORES):
    bf = ml_dtypes.bfloat16
    wqkv_b = np.ascontiguousarray((g1[:, None] * w_qkv).astype(bf))
    wproj_b = np.ascontiguousarray(w_proj.astype(bf))
    wf1_b = np.ascontiguousarray((g2[:, None] * w_ffn1).astype(bf))
    wf2_b = np.ascontiguousarray(w_ffn2.astype(bf))
    cos_t, sin_t = _rope_tables(T)
    cpb = max(1, ncores // z_H.shape[0])
    in_maps, perms = [], []
    for c in range(ncores):
        b, qo = c // cpb, (c % cpb) * Qn
        perm = np.concatenate([np.arange(qo, qo + Qn), np.arange(0, qo), np.arange(qo + Qn, T)])
        perms.append((b, qo))
        in_maps.append(
            dict(
                zz=np.ascontiguousarray(
                    np.stack([z_H[b].T[:, perm], z_L[b].T[:, perm]], axis=1)
                ),
                wqkv=wqkv_b,
                wproj=wproj_b,
                wf1=wf1_b,
                wf2=wf2_b,
                cs=np.ascontiguousarray(
                    np.concatenate([cos_t[perm], sin_t[perm]], axis=1)
                ),
            )
        )
    return in_maps, perms


_CACHED = {}


def kernel(z_H_previous, z_L_current, w_qkv, w_proj, w_ffn1, w_ffn2, g_norm1, g_norm2):
    assert z_H_previous.shape == (B, S, D)
    if "nc" not in _CACHED:
        _CACHED["nc"] = build_bass()
    nc = _CACHED["nc"]
    in_maps, perms = make_in_maps(
        z_H_previous.astype(np.float32),
        z_L_current.astype(np.float32),
        w_qkv.astype(np.float32),
        w_proj.astype(np.float32),
        w_ffn1.astype(np.float32),
        w_ffn2.astype(np.float32),
        g_norm1.astype(np.float32),
        g_norm2.astype(np.float32),
    )
    res = run_bass_kernel_spmd(nc, in_maps, core_ids=list(range(NCORES)))
    out = np.empty((B, S, D), dtype=np.float32)
    for c in range(NCORES):
        b, qo = perms[c]
        out[b, qo : qo + QN, :] = res.results[c]["outt"].T
    return out



# revision 5
# speedup vs baseline: 7.6882x; 7.6882x over previous
"""Trainium2 Bass kernel for a dense transformer block (RMSNorm -> QKV+RoPE ->
attention -> proj -> RMSNorm -> SiLU FFN), sharded over 8 NeuronCores.

The dominant cost in this environment is host<->device transfer over the
axon tunnel (~80 MB/s, ~100ms latency), so the design minimizes shipped
bytes and transfer count:

- Host ships ONE packed bf16 blob per core (~4.3 MB): the core's own
  512-token slice of x = z_H + z_L (natural [tok, D] layout - no host
  transpose), a 1/8 row-shard of each weight matrix (norm gains folded
  in), RoPE tables, and a per-core attention bias row.
- On device, the 8 blobs are AllGathered (fast on-chip links), giving
  every core all 4096 tokens of x and the full weights. Weights are
  never replicated over the tunnel (24 MB total instead of 192 MB).
- Each core computes K/V for all 4096 tokens (both batches) and Q for
  its own 512 tokens, then attends over all 4096 keys with a -30
  pre-softmax bias masking other-batch keys. The bias is shipped as
  data, so the device program is rank-free (pure SPMD, no partition-id).
- proj/norm2/FFN run on the core's own 512 tokens with full weights.
  Output is the core's [D, 512] slice in bf16.

Total tunnel traffic per call: ~37 MB in + 8 MB zero-donation + 8 MB out,
vs ~340 MB for the replicate-everything baseline.
"""

import math
from contextlib import ExitStack

import ml_dtypes
import numpy as np

import concourse.bass as bass
from concourse import bacc
import concourse.mybir as mybir
import concourse.tile as tile
from concourse.bass_utils import run_bass_kernel_spmd
from concourse.masks import make_identity

FP32 = mybir.dt.float32
BF16 = mybir.dt.bfloat16
AF = mybir.ActivationFunctionType
ALU = mybir.AluOpType

B, S, D, F, H, DH = 2, 2048, 1024, 4096, 16, 64
HALF = DH // 2
NCORES = 8
CPB = NCORES // B       # cores per batch
QN = S // CPB           # own query tokens per core (512)
T = B * S               # gathered tokens across all cores (4096)
EPS = 1e-6
ROPE_BASE = 10000.0
P = 128
W = 512                 # matmul moving-dim window
HPW = W // DH           # heads per window (8)
QW = 256                # attention query window
NQW = QN // QW
KD = D // P             # 8
KF = F // P             # 32
TT = T // P             # 32 gathered token tiles
QT = QN // P            # 4
MASK_BIAS = -30.0

# blob layout (elements, all bf16)
OX = 0                                  # x_own   [QN, D]
OWQKV = OX + QN * D                     # wqkv shard [P, 3D] (rows c*128...)
OWPROJ = OWQKV + P * 3 * D              # wproj shard [P, D]
OWF1 = OWPROJ + P * D                   # wf1 shard [P, F]
OWF2 = OWF1 + P * F                     # wf2 shard [F // NCORES, D]
OCS = OWF2 + (F // NCORES) * D          # cos|sin table [S, 2*HALF]
OCSQ = OCS + S * 2 * HALF               # own-query cos|sin [QN, 2*HALF]
OBIAS = OCSQ + QN * 2 * HALF            # key bias row [T]
BLOB = OBIAS + T


def build_bass():
    """Emit the per-core program. All cores run this same NEFF."""
    nc = bacc.Bacc()
    blob = nc.dram_tensor("blob", [BLOB], BF16, kind="ExternalInput")
    outd = nc.dram_tensor("outt", [D, QN], BF16, kind="ExternalOutput")

    with tile.TileContext(nc) as tc:
        with ExitStack() as ctx:
            pool = lambda name, bufs, **kw: ctx.enter_context(
                tc.tile_pool(name=name, bufs=bufs, **kw)
            )
            dram = pool("dram", 1, space="DRAM")
            bounce = dram.tile([BLOB], BF16, tag="bounce")
            gath = dram.tile([NCORES * BLOB], BF16, tag="gath")
            nc.gpsimd.dma_start(bounce, blob[:])
            nc.gpsimd.collective_compute(
                "AllGather",
                ALU.bypass,
                replica_groups=[list(range(NCORES))],
                ins=[bounce.opt()],
                outs=[gath.opt()],
            )
            gap = gath[:]
            bap = blob[:]

            def gv(off, dims):
                return bass.AP(
                    tensor=gap.tensor, offset=gap.offset + off,
                    ap=[list(d) for d in dims],
                )

            def bv(off, dims):
                return bass.AP(
                    tensor=bap.tensor, offset=bap.offset + off,
                    ap=[list(d) for d in dims],
                )

            # ---- persistent small tiles ----
            psingle = pool("psingle", 1)
            ident = psingle.tile([P, P], BF16)
            make_identity(nc, ident)
            ones_col = psingle.tile([P, 1], BF16)
            nc.vector.memset(ones_col, 1.0)
            ones_row = psingle.tile([1, P], FP32)
            nc.vector.memset(ones_row, 1.0)
            eps_t = psingle.tile([P, 1], FP32)
            nc.vector.memset(eps_t, EPS)
            zero_t = psingle.tile([P, 1], FP32)
            nc.vector.memset(zero_t, 0.0)

            pqT = pool("pqT", 1)
            qT = pqT.tile([P, KD, QN], BF16, tag="qT")        # roped q, [dh, hc, tok]
            pattn = pool("pattn", 1)
            attn = pattn.tile([P, KD, QN], BF16, tag="attn")  # attn out, [dh, hc, tok]
            pxres = pool("pxres", 1)
            xres = pxres.tile([P, KD, QN], FP32, tag="xres")  # own x -> residual accum
            pbias = pool("pbias", 1)
            bias_f = pbias.tile([P, TT], FP32, tag="biasf")   # per-ktok exp bias

            # load bias row: token t = kt*128 + p
            bias_b = pbias.tile([P, TT], BF16, tag="biasb")
            nc.sync.dma_start(bias_b, bv(OBIAS, [[1, P], [P, TT]]))
            nc.vector.tensor_copy(bias_f, bias_b)

            ps_mm = pool("ps_mm", 3, space="PSUM")
            ps_tp = pool("ps_tp", 1, space="PSUM")
            ps_st = pool("ps_st", 1, space="PSUM")

            def norm_tile(px, xt, ptmp, pst):
                """xt [P, D] bf16 -> ht [P, D] bf16 (rmsnorm, gain folded in w)."""
                sq = ptmp.tile([P, D], BF16, tag="sq")
                ssq = pst.tile([P, 1], FP32, tag="ssq")
                nc.vector.tensor_mul(sq, xt, xt)
                nc.vector.tensor_reduce(ssq, sq, mybir.AxisListType.X, ALU.add)
                srt = pst.tile([P, 1], FP32, tag="srt")
                nc.scalar.activation(srt, ssq, AF.Sqrt, bias=eps_t, scale=1.0 / D)
                rstd = pst.tile([P, 1], FP32, tag="rstd")
                nc.vector.reciprocal(rstd, srt)
                ht = px.tile([P, D], BF16, tag="ht")
                nc.vector.tensor_scalar_mul(ht, xt, rstd)
                return ht

            def rope_window(ps, cs_src, prope, ptmp):
                """ps [P, HPW, DH] psum fp32 -> rop [P, W] bf16 (roped)."""
                csb = prope.tile([P, HPW, 2 * HALF], BF16, tag="csb")
                nc.sync.dma_start(csb, cs_src)
                csf = prope.tile([P, HPW, 2 * HALF], FP32, tag="csf")
                nc.vector.tensor_copy(csf, csb)
                crep = csf[:, :, 0:HALF]
                srep = csf[:, :, HALF : 2 * HALF]
                rop = ptmp.tile([P, W], BF16, tag="rop")
                rop3 = rop.rearrange("p (h j) -> p h j", j=DH)
                ta = prope.tile([P, HPW, HALF], BF16, tag="ta")
                tb = prope.tile([P, HPW, HALF], BF16, tag="tb")
                nc.vector.tensor_mul(ta, ps[:, :, 0:HALF], crep)
                nc.vector.tensor_mul(tb, ps[:, :, HALF:DH], srep)
                nc.vector.tensor_sub(rop3[:, :, 0:HALF], ta, tb)
                tc2 = prope.tile([P, HPW, HALF], BF16, tag="ta")
                td = prope.tile([P, HPW, HALF], BF16, tag="tb")
                nc.vector.tensor_mul(tc2, ps[:, :, HALF:DH], crep)
                nc.vector.tensor_mul(td, ps[:, :, 0:HALF], srep)
                nc.vector.tensor_add(rop3[:, :, HALF:DH], tc2, td)
                return rop

            with ExitStack() as c1:
                pool1 = lambda name, bufs, **kw: c1.enter_context(
                    tc.tile_pool(name=name, bufs=bufs, **kw)
                )
                pkT = pool1("pkT", 1)
                kT = pkT.tile([P, KD, T], BF16, tag="kT")     # roped k, [dh, hc, tok]
                pv = pool1("pv", 1)
                v65 = pv.tile([P, TT, H, DH + 1], BF16, tag="v65")
                nc.vector.memset(v65[:, :, :, DH : DH + 1], 1.0)
                ps_kv = pool1("ps_kv", 2, space="PSUM")

                # ---- K pass then V pass over all gathered tokens ----
                # each pass holds 2 weight windows (1024 cols) resident and
                # recomputes the hidden tile per 128-token tile.
                for vpass in range(2):  # 0: K cols, 1: V cols
                    with ExitStack() as c2:
                        pool2 = lambda name, bufs, **kw: c2.enter_context(
                            tc.tile_pool(name=name, bufs=bufs, **kw)
                        )
                        pw = pool2("pw", 1)
                        pxt = pool2("pxt", 2)
                        pht = pool2("pht", 2)
                        phid = pool2("phid", 2)
                        prope = pool2("prope", 2)
                        ptmp = pool2("ptmp", 2)
                        pst = pool2("pst", 2)
                        wts = []
                        for wi in range(2):
                            wt = pw.tile([P, KD, W], BF16, tag=f"w{wi}")
                            off = OWQKV + (1 + vpass) * D + wi * W
                            nc.sync.dma_start(
                                wt, gv(off, [[3 * D, P], [BLOB, NCORES], [1, W]])
                            )
                            wts.append(wt)
                        for tt in range(TT):
                            ch, r0 = tt // 4, (tt % 4) * P
                            xt = pxt.tile([P, D], BF16, tag="xt")
                            nc.gpsimd.dma_start(
                                xt, gv(ch * BLOB + OX + r0 * D, [[D, P], [1, D]])
                            )
                            ht = norm_tile(pht, xt, ptmp, pst)
                            hidt = phid.tile([P, KD, P], BF16, tag="hidt")
                            for c2i in range(KD):
                                tp = ps_tp.tile([P, P], BF16, tag="tpps")
                                nc.tensor.transpose(
                                    tp, ht[:, c2i * P : (c2i + 1) * P], ident
                                )
                                nc.vector.tensor_copy(hidt[:, c2i, :], tp)
                            for wi in range(2):
                                ps = ps_kv.tile([P, W], FP32, tag="kvps")
                                for dc in range(KD):
                                    nc.tensor.matmul(
                                        ps,
                                        hidt[:, dc, :],
                                        wts[wi][:, dc, :],
                                        start=(dc == 0),
                                        stop=(dc == KD - 1),
                                    )
                                ps3 = ps.rearrange("p (h j) -> p h j", j=DH)
                                if vpass == 1:
                                    h0 = wi * HPW
                                    nc.vector.tensor_copy(
                                        v65[:, tt, h0 : h0 + HPW, 0:DH], ps3
                                    )
                                else:
                                    cs_src = gv(
                                        OCS + ((tt * P) % S) * 2 * HALF,
                                        [[2 * HALF, P], [0, HPW], [1, 2 * HALF]],
                                    )
                                    rop = rope_window(ps3, cs_src, prope, ptmp)
                                    for c2i in range(W // P):
                                        tp = ps_tp.tile([P, P], BF16, tag="tpps")
                                        nc.tensor.transpose(
                                            tp, rop[:, c2i * P : (c2i + 1) * P], ident
                                        )
                                        gc = wi * (W // P) + c2i
                                        nc.vector.tensor_copy(
                                            kT[:, gc, tt * P : (tt + 1) * P], tp
                                        )

                # ---- Q pass: own 512 tokens ----
                with ExitStack() as c2:
                    pool2 = lambda name, bufs, **kw: c2.enter_context(
                        tc.tile_pool(name=name, bufs=bufs, **kw)
                    )
                    phq = pool2("phq", 1)
                    hqT = phq.tile([P, KD, QN], BF16, tag="hqT")
                    pxt = pool2("pxt", 2)
                    pht = pool2("pht", 2)
                    prope = pool2("prope", 2)
                    ptmp = pool2("ptmp", 2)
                    pst = pool2("pst", 2)
                    pwq = pool2("pwq", 1)
                    for qt in range(QT):
                        xt = pxt.tile([P, D], BF16, tag="xt")
                        nc.gpsimd.dma_start(
                            xt, bv(OX + qt * P * D, [[D, P], [1, D]])
                        )
                        # transpose own x into residual tile (fp32)
                        for c2i in range(KD):
                            tp = ps_tp.tile([P, P], BF16, tag="tpps")
                            nc.tensor.transpose(
                                tp, xt[:, c2i * P : (c2i + 1) * P], ident
                            )
                            nc.vector.tensor_copy(
                                xres[:, c2i, qt * P : (qt + 1) * P], tp
                            )
                        ht = norm_tile(pht, xt, ptmp, pst)
                        for c2i in range(KD):
                            tp = ps_tp.tile([P, P], BF16, tag="tpps")
                            nc.tensor.transpose(
                                tp, ht[:, c2i * P : (c2i + 1) * P], ident
                            )
                            nc.vector.tensor_copy(
                                hqT[:, c2i, qt * P : (qt + 1) * P], tp
                            )
                    for wi in range(2):
                        wt = pwq.tile([P, KD, W], BF16, tag="wq")
                        nc.sync.dma_start(
                            wt, gv(OWQKV + wi * W, [[3 * D, P], [BLOB, NCORES], [1, W]])
                        )
                        for qt in range(QT):
                            ps = ps_mm.tile([P, W], FP32, tag="mmps")
                            for dc in range(KD):
                                nc.tensor.matmul(
                                    ps,
                                    hqT[:, dc, qt * P : (qt + 1) * P],
                                    wt[:, dc, :],
                                    start=(dc == 0),
                                    stop=(dc == KD - 1),
                                )
                            ps3 = ps.rearrange("p (h j) -> p h j", j=DH)
                            cs_src = bv(
                                OCSQ + qt * P * 2 * HALF,
                                [[2 * HALF, P], [0, HPW], [1, 2 * HALF]],
                            )
                            rop = rope_window(ps3, cs_src, prope, ptmp)
                            for c2i in range(W // P):
                                tp = ps_tp.tile([P, P], BF16, tag="tpps")
                                nc.tensor.transpose(
                                    tp, rop[:, c2i * P : (c2i + 1) * P], ident
                                )
                                gc = wi * (W // P) + c2i
                                nc.vector.tensor_copy(
                                    qT[:, gc, qt * P : (qt + 1) * P], tp
                                )

                # ---- attention over all 4096 keys ----
                with ExitStack() as c2:
                    pool2 = lambda name, bufs, **kw: c2.enter_context(
                        tc.tile_pool(name=name, bufs=bufs, **kw)
                    )
                    pex = pool2("pex", 2)
                    phead = pool2("phead", 2)
                    for h in range(H):
                        hc, hp = h // 2, (h % 2) * DH
                        for qw in range(NQW):
                            qsl = qT[hp : hp + DH, hc, qw * QW : (qw + 1) * QW]
                            ex = pex.tile([P, TT, QW], BF16, tag="ex")
                            for kt in range(TT):
                                pss = ps_mm.tile([P, QW], FP32, tag="mmps")
                                nc.tensor.matmul(
                                    pss,
                                    kT[hp : hp + DH, hc, kt * P : (kt + 1) * P],
                                    qsl,
                                    start=True,
                                    stop=True,
                                )
                                nc.scalar.activation(
                                    ex[:, kt, :], pss, AF.Exp,
                                    bias=bias_f[:, kt : kt + 1],
                                    scale=1.0 / math.sqrt(DH),
                                )
                            pso = ps_mm.tile([DH + 1, QW], FP32, tag="mmps")
                            for kt in range(TT):
                                nc.tensor.matmul(
                                    pso,
                                    v65[:, kt, h, :],
                                    ex[:, kt, :],
                                    start=(kt == 0),
                                    stop=(kt == TT - 1),
                                )
                            rc = phead.tile([1, QW], FP32, tag="rcrow")
                            nc.vector.reciprocal(rc, pso[DH : DH + 1, :])
                            rb = ps_tp.tile([DH, QW], FP32, tag="tpps")
                            nc.tensor.matmul(
                                rb, ones_row[0:1, 0:DH], rc, start=True, stop=True
                            )
                            rbs = phead.tile([DH, QW], FP32, tag="rbsb")
                            nc.vector.tensor_copy(rbs, rb)
                            nc.vector.tensor_mul(
                                attn[hp : hp + DH, hc, qw * QW : (qw + 1) * QW],
                                pso[0:DH, :],
                                rbs,
                            )

            # ---- proj + residual (into xres in place) ----
            with ExitStack() as c1:
                pool1 = lambda name, bufs, **kw: c1.enter_context(
                    tc.tile_pool(name=name, bufs=bufs, **kw)
                )
                pwp = pool1("pwp", 2)
                for dt in range(KD):
                    wp = pwp.tile([P, KD, P], BF16, tag="wp")
                    nc.sync.dma_start(
                        wp, gv(OWPROJ + dt * P, [[D, P], [BLOB, NCORES], [1, P]])
                    )
                    ps = ps_mm.tile([P, QN], FP32, tag="mmps")
                    for ac in range(KD):
                        nc.tensor.matmul(
                            ps, wp[:, ac, :], attn[:, ac, :],
                            start=(ac == 0), stop=(ac == KD - 1),
                        )
                    nc.vector.tensor_add(xres[:, dt, :], ps, xres[:, dt, :])

            # ---- norm2 + FFN ----
            with ExitStack() as c1:
                pool1 = lambda name, bufs, **kw: c1.enter_context(
                    tc.tile_pool(name=name, bufs=bufs, **kw)
                )
                psq2 = pool1("psq2", 2)
                prow = pool1("prow", 1)
                prstd = pool1("prstd", 1)
                ph2 = pool1("ph2", 1)
                st2 = ps_st.tile([1, QN], FP32, tag="stps")
                for dt in range(KD):
                    sq2 = psq2.tile([P, QN], BF16, tag="sq2")
                    nc.vector.tensor_mul(sq2, xres[:, dt, :], xres[:, dt, :])
                    nc.tensor.matmul(
                        st2, ones_col, sq2, start=(dt == 0), stop=(dt == KD - 1)
                    )
                rows2 = prow.tile([33, QN], FP32, tag="srow")
                nc.scalar.activation(
                    rows2[32:33, :], st2, AF.Sqrt, bias=eps_t[32:33], scale=1.0 / D
                )
                nc.vector.reciprocal(rows2[0:1, :], rows2[32:33, :])
                rstd2 = prstd.tile([P, QN], BF16, tag="rstd2")
                rb2 = ps_st.tile([P, QN], FP32, tag="stps")
                nc.tensor.matmul(rb2, ones_row, rows2[0:1, :], start=True, stop=True)
                nc.vector.tensor_copy(rstd2, rb2)
                h2 = ph2.tile([P, KD, QN], BF16, tag="h2")
                for dt in range(KD):
                    nc.vector.tensor_mul(h2[:, dt, :], xres[:, dt, :], rstd2)

                psil = pool1("psil", 1)
                pw1 = pool1("pw1", 2)
                sil = psil.tile([P, KF, QN], BF16, tag="sil")
                for ft in range(KF):
                    w1t = pw1.tile([P, KD, P], BF16, tag="w1t")
                    nc.sync.dma_start(
                        w1t, gv(OWF1 + ft * P, [[F, P], [BLOB, NCORES], [1, P]])
                    )
                    ps = ps_mm.tile([P, QN], FP32, tag="mmps")
                    for dc in range(KD):
                        nc.tensor.matmul(
                            ps, w1t[:, dc, :], h2[:, dc, :],
                            start=(dc == 0), stop=(dc == KD - 1),
                        )
                    nc.scalar.activation(sil[:, ft, :], ps, AF.Silu, bias=zero_t)
                pw2 = pool1("pw2", 2)
                pout = pool1("pout", 2)
                FQ = F // NCORES // P  # f-tiles per gathered chunk (4)
                for dt in range(KD):
                    w2t = pw2.tile([P, NCORES, FQ, P], BF16, tag="w2t")
                    for cc in range(NCORES):
                        nc.sync.dma_start(
                            w2t[:, cc, :, :],
                            gv(
                                cc * BLOB + OWF2 + dt * P,
                                [[D, P], [P * D, FQ], [1, P]],
                            ),
                        )
                    ps = ps_mm.tile([P, QN], FP32, tag="mmps")
                    for fc in range(KF):
                        nc.tensor.matmul(
                            ps,
                            w2t[:, fc // FQ, fc % FQ, :],
                            sil[:, fc, :],
                            start=(fc == 0),
                            stop=(fc == KF - 1),
                        )
                    ot = pout.tile([P, QN], BF16, tag="outsb")
                    otf = pout.tile([P, QN], FP32, tag="outf")
                    nc.vector.tensor_add(otf, ps, xres[:, dt, :])
                    nc.vector.tensor_copy(ot, otf)
                    nc.sync.dma_start(outd[dt * P : (dt + 1) * P, :], ot)

    nc.finalize()
    return nc


def _rope_tables():
    inv = ROPE_BASE ** (-np.arange(HALF, dtype=np.float64) / HALF)
    fr = np.arange(S, dtype=np.float64)[:, None] * inv[None, :]
    cs = np.concatenate([np.cos(fr), np.sin(fr)], axis=1)
    return cs.astype(ml_dtypes.bfloat16)


def make_in_maps(z_H, z_L, w_qkv, w_proj, w_ffn1, w_ffn2, g1, g2):
    bf = ml_dtypes.bfloat16
    x = (np.asarray(z_H, np.float32) + np.asarray(z_L, np.float32)).astype(bf)
    wqkv_b = (np.asarray(g1, np.float32)[:, None] * np.asarray(w_qkv, np.float32)).astype(bf)
    wproj_b = np.asarray(w_proj, np.float32).astype(bf)
    wf1_b = (np.asarray(g2, np.float32)[:, None] * np.asarray(w_ffn1, np.float32)).astype(bf)
    wf2_b = np.asarray(w_ffn2, np.float32).astype(bf)
    cs = _rope_tables()
    FR = F // NCORES
    in_maps, perms = [], []
    for c in range(NCORES):
        b, qo = c // CPB, (c % CPB) * QN
        blob = np.empty(BLOB, bf)
        blob[OX : OX + QN * D] = x[b, qo : qo + QN].ravel()
        blob[OWQKV : OWQKV + P * 3 * D] = wqkv_b[c * P : (c + 1) * P].ravel()
        blob[OWPROJ : OWPROJ + P * D] = wproj_b[c * P : (c + 1) * P].ravel()
        blob[OWF1 : OWF1 + P * F] = wf1_b[c * P : (c + 1) * P].ravel()
        blob[OWF2 : OWF2 + FR * D] = wf2_b[c * FR : (c + 1) * FR].ravel()
        blob[OCS : OCS + S * DH] = cs.ravel()
        blob[OCSQ : OCSQ + QN * DH] = cs[qo : qo + QN].ravel()
        bias = np.zeros(T, np.float32)
        other = slice(S, T) if b == 0 else slice(0, S)
        bias[other] = MASK_BIAS
        blob[OBIAS : OBIAS + T] = bias.astype(bf)
        in_maps.append(dict(blob=blob))
        perms.append((b, qo))
    return in_maps, perms


_CACHED = {}


def kernel(z_H_previous, z_L_current, w_qkv, w_proj, w_ffn1, w_ffn2, g_norm1, g_norm2):
    assert z_H_previous.shape == (B, S, D)
    if "nc" not in _CACHED:
        _CACHED["nc"] = build_bass()
    nc = _CACHED["nc"]
    in_maps, perms = make_in_maps(
        z_H_previous, z_L_current, w_qkv, w_proj, w_ffn1, w_ffn2, g_norm1, g_norm2
    )
    res = run_bass_kernel_spmd(nc, in_maps, core_ids=list(range(NCORES)))
    out = np.empty((B, S, D), dtype=np.float32)
    for c in range(NCORES):
        b, qo = perms[c]
        out[b, qo : qo + QN, :] = res.results[c]["outt"].astype(np.float32).T
    return out
